# revision 25
# baseline (speedup 1.0000x reference)
"""v2: pair-symmetry kernel. w(p,s)*|dsal(p,s)| is symmetric under
(p,s) -> (p+s,-s), so each of the 60 shift pairs is computed once on an
extended domain (rows -6..353, cols -5..356) and accumulated twice:
once directly (A) and once re-shifted (per-sy groups G, row-shifted via DMA).
Accumulation runs on the TensorEngine as identity-matmuls into PSUM (fp32).

Layout: 120 partitions x 3 payload rows (global row 3p-6+j), per-channel
local window 13 rows x 372 cols fp16, all 4 channels in one tile (single
big sub/square instructions cover rgb).
"""

import numpy as np

H = W = 352
RADIUS = 5
NP = 120                 # partitions; payload rows 3p-6 .. 3p-4
PADW2 = W + 20           # 372 : cols idx t <-> global col t-10
LROWS = 13               # local rows k <-> global row 3p-11+k
CH = LROWS * PADW2       # 4836 elements per channel
PW = W + 2 * RADIUS      # 362 : P/ssq domain, col q <-> global col q-5
N_CORES = 8
USE_PE = True

# (p0, np, k0, nk) valid local rows: global row 3p-11+k in [0,352)
REGIONS2 = [
    (0, 1, 11, 2),
    (1, 1, 8, 5),
    (2, 1, 5, 8),
    (3, 1, 2, 11),
    (4, 113, 0, 13),
    (117, 1, 0, 12),
    (118, 1, 0, 9),
    (119, 1, 0, 6),
]

_CACHE = {}


def _build_kernel():
    from contextlib import ExitStack

    import concourse.bass as bass
    import concourse.tile as tile
    from concourse import bacc, mybir

    f16 = mybir.dt.float16
    f32 = mybir.dt.float32
    i16 = mybir.dt.int16
    Alu = mybir.AluOpType
    Act = mybir.ActivationFunctionType

    nc = bacc.Bacc(
        "TRN2",
        debug=False,
        enable_asserts=False,
        target_bir_lowering=False,
        num_devices=1,
        enable_partition_id=False,
    )
    # host-padded fp16 inputs: row r <-> global row r-11, col t <-> global t-10
    pred_d = nc.dram_tensor("pred16", [370, PADW2], f16, kind="ExternalInput")
    feat_d = nc.dram_tensor("feat16", [3, 370, PADW2], f16, kind="ExternalInput")
    out_d = nc.dram_tensor("partial", [NP, 2], f32, kind="ExternalOutput")

    with tile.TileContext(nc) as tc, ExitStack() as ctx:
        persist = ctx.enter_context(tc.tile_pool(name="persist", bufs=1))

        # all 4 channels in one tile; odd-shifted copy of the rgb channels
        ch4 = persist.tile([NP, 4, LROWS, PADW2], f16, tag="ch4")
        cho = persist.tile([NP, 3, LROWS, PADW2], f16, tag="cho")

        # sal (c=3) first so the mask pipeline overlaps the rgb loads
        for c in (3, 0, 1, 2):
            src_ap = pred_d.ap() if c == 3 else feat_d.ap()[c]
            src = bass.AP(
                tensor=src_ap.tensor,
                offset=src_ap.offset,
                ap=[[3 * PADW2, NP], [PADW2, LROWS], [1, PADW2]],
            )
            nc.sync.dma_start(out=ch4[:, c, :, :], in_=src)
        ch4f = ch4[:].rearrange("p c a b -> p (c a b)")
        chof = cho[:].rearrange("p c a b -> p (c a b)")
        for c in range(3):
            nc.scalar.copy(
                out=chof[:, c * CH : (c + 1) * CH],
                in_=ch4f[:, c * CH + 1 : (c + 1) * CH + 1],
            )

        zeros = persist.tile([1, 5 * PADW2], f16, tag="zeros")
        nc.vector.memset(zeros[:], 0.0)

        # ---- contour mask (in P-column coords, [NP, 3, 362]) ----
        sal = ch4[:, 3]
        salf = sal.rearrange("p a b -> p (a b)")
        with tc.tile_pool(name="maskpool", bufs=1) as mp:
            lbl = mp.tile([NP, LROWS, PADW2], f16, tag="lbl")
            nc.vector.tensor_scalar(
                out=lbl.rearrange("p a b -> p (a b)"), in0=salf,
                scalar1=0.5, scalar2=None, op0=Alu.is_gt,
            )
            u = mp.tile([NP, LROWS, PADW2], f16, tag="u")
            nc.vector.tensor_scalar(
                out=u.rearrange("p a b -> p (a b)"), in0=salf,
                scalar1=0.5, scalar2=None, op0=Alu.is_le,
            )
            # invalidate u outside the image: pad cols, then pad rows the
            # +-2 pool windows can reach (k in [3,10])
            nc.vector.memset(u[:, :, 0:10], 0.0)
            nc.vector.memset(u[:, :, 362:372], 0.0)
            nc.vector.memset(u[0:1, 3:11, :], 0.0)
            nc.sync.dma_start(out=u[1:2, 3:8, :], in_=zeros[:, 0 : 5 * PADW2])
            nc.sync.dma_start(out=u[2:3, 3:5, :], in_=zeros[:, 0 : 2 * PADW2])
            nc.sync.dma_start(out=u[118:119, 9:11, :], in_=zeros[:, 0 : 2 * PADW2])
            nc.sync.dma_start(out=u[119:120, 6:11, :], in_=zeros[:, 0 : 5 * PADW2])

            ladA = mp.tile([NP, 6, PADW2], f16, tag="ladA")
            ladB = mp.tile([NP, 3, PADW2], f16, tag="ladB")
            rowm = mp.tile([NP, 3, PADW2], f16, tag="rowm")
            ladC = mp.tile([NP, 3, PW + 2], f16, tag="ladC")
            ladD = mp.tile([NP, 3, PW], f16, tag="ladD")
            dil = persist.tile([NP, 3, PW], f16, tag="dil")
            umax = persist.tile([NP, 3, PW], f16, tag="umax")
            for eng, srcb, dstb in ((nc.vector, lbl, dil), (nc.vector, u, umax)):
                eng.tensor_max(ladA[:], srcb[:, 3:9, :], srcb[:, 4:10, :])
                eng.tensor_max(ladB[:], ladA[:, 0:3, :], ladA[:, 2:5, :])
                eng.tensor_max(rowm[:], ladB[:], srcb[:, 7:10, :])
                # cols: dil[q] = max rowm[t=q+3..q+7]
                eng.tensor_max(ladC[:], rowm[:, :, 3 : 3 + PW + 2], rowm[:, :, 4 : 4 + PW + 2])
                eng.tensor_max(ladD[:], ladC[:, :, 0:PW], ladC[:, :, 2 : 2 + PW])
                eng.tensor_max(dstb[:], ladD[:], rowm[:, :, 7 : 7 + PW])
        mask = persist.tile([NP, 3, PW], f16, tag="mask")
        nc.vector.scalar_tensor_tensor(
            out=mask[:].rearrange("p a b -> p (a b)"),
            in0=dil[:].rearrange("p a b -> p (a b)"),
            scalar=-1.0,
            in1=umax[:].rearrange("p a b -> p (a b)"),
            op0=Alu.add, op1=Alu.add,
        )
        # zero mask outside the image: pad cols, pad partitions, junk rows
        nc.vector.memset(mask[:, :, 0:RADIUS], 0.0)
        nc.vector.memset(mask[:, :, RADIUS + W : PW], 0.0)
        nc.vector.memset(mask[0:2, :, :], 0.0)
        nc.sync.dma_start(out=mask[119:120, 1:3, :], in_=zeros[:, 0 : 2 * PW])

        # ---- identity for PE accumulation ----
        ident = persist.tile([NP, NP], f16, tag="ident")
        rowidx = persist.tile([NP, NP], i16, tag="rowidx")
        pidx = persist.tile([NP, 1], mybir.dt.int32, tag="pidx")
        pidxf = persist.tile([NP, 1], f32, tag="pidxf")
        nc.gpsimd.iota(rowidx[:], pattern=[[1, NP]], base=0, channel_multiplier=0)
        nc.gpsimd.iota(pidx[:], pattern=[[1, 1]], base=0, channel_multiplier=1)
        nc.vector.tensor_copy(out=pidxf[:], in_=pidx[:])
        nc.vector.tensor_scalar(
            out=ident[:], in0=rowidx[:], scalar1=pidxf[:], scalar2=None,
            op0=Alu.is_equal,
        )

        pp = ctx.enter_context(tc.tile_pool(name="ps", bufs=1, space="PSUM"))
        psA = pp.tile([NP, 3, 512], f32, tag="psA")
        psG = pp.tile([NP, 3, 512], f32, tag="psG")

        tmp = ctx.enter_context(tc.tile_pool(name="tmp", bufs=3))
        Gs = [persist.tile([NP, 3, W], f16, tag=f"Gs{sy}", name=f"Gs{sy}")
              for sy in range(1, 6)]

        # half set grouped by sy; sy=0 last so every G-merge chain
        # overlaps later groups and the kernel tail is short
        groups = [(sy, list(range(-5, 6))) for sy in range(5, 0, -1)] + [
            (0, [sx for sx in range(1, 6)])
        ]
        pair_idx = 0
        first_G = {}
        for (sy, sxs) in groups:
            for gi, sx in enumerate(sxs):
                # window / center views (parity-aligned)
                off_par = (5 + sx) % 2
                if off_par == 0:
                    winr = ch4[:, 0:3, 5 + sy : 8 + sy, 5 + sx : 5 + sx + PW]
                else:
                    winr = cho[:, 0:3, 5 + sy : 8 + sy, 4 + sx : 4 + sx + PW]
                ctr = cho[:, 0:3, 5:8, 4 : 4 + PW]

                d4 = tmp.tile([NP, 3, 3, PW], f16, tag="d4")
                nc.vector.tensor_sub(d4[:], winr, ctr)
                q4 = tmp.tile([NP, 3, 3, PW], f16, tag="q4")
                nc.scalar.activation(out=q4[:], in_=d4[:], func=Act.Square)
                s1 = tmp.tile([NP, 3, PW], f16, tag="s1")
                nc.vector.tensor_add(s1[:], q4[:, 0], q4[:, 1])
                ssq = tmp.tile([NP, 3, PW], f16, tag="ssq")
                nc.vector.tensor_add(ssq[:], s1[:], q4[:, 2])
                wgt = tmp.tile([NP, 3, PW], f16, tag="wgt")
                nc.scalar.activation(out=wgt[:], in_=ssq[:], func=Act.Exp,
                                     scale=-200.0)

                dsal = tmp.tile([NP, 3, PW], f16, tag="dsal")
                nc.gpsimd.tensor_sub(
                    dsal[:],
                    ch4[:, 3, 5 + sy : 8 + sy, 5 + sx : 5 + sx + PW],
                    ch4[:, 3, 5:8, 5 : 5 + PW],
                )
                adsal = tmp.tile([NP, 3, PW], f16, tag="adsal")
                nc.vector.tensor_scalar(
                    out=adsal[:].bitcast(mybir.dt.uint16),
                    in0=dsal[:].bitcast(mybir.dt.uint16),
                    scalar1=0x7FFF, scalar2=None, op0=Alu.bitwise_and,
                )
                P = tmp.tile([NP, 3, PW], f16, tag="P")
                nc.vector.tensor_mul(P[:], wgt[:], adsal[:])

                # direct accumulation: psA[:, j, :] += P[:, j, :]
                for j in range(3):
                    nc.tensor.matmul(
                        out=psA[:, j, 0:PW], lhsT=ident[:], rhs=P[:, j, :],
                        start=(pair_idx == 0), stop=False,
                        skip_group_check=True,
                    )
                # mirror accumulation
                if sy == 0:
                    # no row shift: psA grid cols += P shifted by -sx
                    last_write = gi == len(sxs) - 1
                    for j in range(3):
                        nc.tensor.matmul(
                            out=psA[:, j, RADIUS : RADIUS + W],
                            lhsT=ident[:],
                            rhs=P[:, j, RADIUS - sx : RADIUS - sx + W],
                            start=False, stop=(last_write and j == 2),
                            skip_group_check=True,
                        )
                else:
                    for j in range(3):
                        nc.tensor.matmul(
                            out=psG[:, j, 0:W], lhsT=ident[:],
                            rhs=P[:, j, RADIUS - sx : RADIUS - sx + W],
                            start=(gi == 0), stop=(gi == len(sxs) - 1),
                            skip_group_check=True,
                        )
                pair_idx += 1
            if sy > 0:
                # evacuate this group's PSUM, row-shift via DMA, merge into psA
                nc.vector.tensor_copy(out=Gs[sy - 1][:], in_=psG[:, :, 0:W])
                gsh = persist.tile([NP, 3, W], f16, tag=f"gsh{sy}",
                                   name=f"gsh{sy}")
                nc.vector.memset(gsh[0:2, :, :], 0.0)
                for j in range(3):
                    jp = (j - sy) % 3
                    dp = (j - sy - jp) // 3
                    nc.sync.dma_start(
                        out=gsh[2:NP, j, :],
                        in_=Gs[sy - 1][2 + dp : NP + dp, jp, :],
                    )
                for j in range(3):
                    nc.tensor.matmul(
                        out=psA[:, j, RADIUS : RADIUS + W],
                        lhsT=ident[:],
                        rhs=gsh[:, j, :],
                        start=False, stop=False, skip_group_check=True,
                    )

        lm = persist.tile([NP, 3, PW], f16, tag="lm")
        nc.vector.tensor_copy(out=lm[:], in_=psA[:, :, 0:PW])

        # ---- masked partial sums ----
        sums = persist.tile([NP, 2], f32, tag="sums")
        scratch = persist.tile([NP, 3, PW], f16, tag="scratch")
        nc.vector.tensor_mul(
            scratch[:].rearrange("p a b -> p (a b)"),
            lm[:].rearrange("p a b -> p (a b)"),
            mask[:].rearrange("p a b -> p (a b)"),
        )
        nc.vector.tensor_reduce(
            out=sums[:, 0:1], in_=scratch[:].rearrange("p a b -> p (a b)"),
            axis=mybir.AxisListType.X, op=Alu.add,
        )
        nc.vector.tensor_reduce(
            out=sums[:, 1:2], in_=mask[:].rearrange("p a b -> p (a b)"),
            axis=mybir.AxisListType.X, op=Alu.add,
        )
        nc.sync.dma_start(out=out_d.ap(), in_=sums[:])

    nc.compile()
    return nc


def kernel(pred, feat):
    import os

    # A stale PJRT compilation-cache hit was observed to return a bad
    # executable (NaN result); force a fresh compile per process.
    os.environ.setdefault("JAX_ENABLE_COMPILATION_CACHE", "false")
    try:
        import jax

        jax.config.update("jax_enable_compilation_cache", False)
    except Exception:
        pass

    if "nc" not in _CACHE:
        _CACHE["nc"] = _build_kernel()
    nc = _CACHE["nc"]
    from concourse.bass_utils import run_bass_kernel_spmd

    pred = np.asarray(pred, dtype=np.float32).reshape(N_CORES, H, W)
    feat = np.asarray(feat, dtype=np.float32).reshape(N_CORES, 3, H, W)
    predp = np.zeros((N_CORES, 370, PADW2), np.float16)
    predp[:, 11:363, 10:362] = pred.astype(np.float16)
    featp = np.zeros((N_CORES, 3, 370, PADW2), np.float16)
    featp[:, :, 11:363, 10:362] = feat.astype(np.float16)
    in_maps = [
        {"pred16": np.ascontiguousarray(predp[i]),
         "feat16": np.ascontiguousarray(featp[i])}
        for i in range(N_CORES)
    ]
    res = run_bass_kernel_spmd(nc, in_maps, core_ids=list(range(N_CORES)))
    _CACHE["last_results"] = res
    tot = np.zeros(2, np.float64)
    for r in res.results:
        tot += r["partial"].astype(np.float64).sum(axis=0)
    loss = tot[0] / (tot[1] + 1e-6)
    return np.array(loss, dtype=np.float32)
